# revision 12
# baseline (speedup 1.0000x reference)
"""Trainium2 Bass kernel for BaselineMoE (top-6-of-32 routed experts + 2 shared).

Strategy (8 NeuronCores, expert-parallel per the sharding hint):
  - Host computes the (cheap) router softmax/top-k from the actual inputs,
    gathers each expert's tokens into a padded, transposed buffer, and deals
    the 32 routed experts across 8 cores x 4 slots, balancing per-core load.
  - Each core runs a dense SwiGLU MLP (gate/up/down, sigmoid(gate)*up) for its
    4 routed experts on the pre-gathered tokens in fp8e4 with DoubleRow
    matmuls (2 contraction rows per PE cell); per-token top-k gate weights are
    applied on-device during PSUM evacuation. PSUM accumulation stays f32.
  - The 2 shared experts are split across core halves (cores 0-3 run shared
    expert 0, cores 4-7 expert 1, each on a 512-token shard) as a 5th slot:
    the gate projection runs fp8-DoubleRow (the sigmoid's <=0.25 slope damps
    fp8 error), while the up/down projections run bf16 — they set the output
    magnitude and need the mantissa.
  - The PE stream is software-pipelined at PSUM-group granularity: slot s's
    16 down-projection groups interleave 1:1 with slot s+1's 16 gate/up
    groups (independent work), so projection-phase transitions never drain
    the PE.
  - Every DRAM tensor is partition-major ([128, ...]), so each load/store is
    one DMA with long contiguous runs per partition (8-16KB). Loads issue on
    the sync HWDGE queue, stores on the scalar queue, so a store waiting for
    compute never blocks the next slot's weight prefetch. All slots' gate
    vectors load as one broadcast DMA.
  - Routed outputs come back fp8 (scaled by S_Y), shared bf16; the host
    scatter-adds them into the residual stream in f32.

Capacities (per-slot token counts) are computed from the actual routing at
call time, so the emitted program adapts to the input.
"""

from contextlib import ExitStack

import numpy as np
import ml_dtypes

import concourse.bacc as bacc
import concourse.tile as tile
import concourse.mybir as mybir
from concourse.bass_utils import run_bass_kernel_spmd

H = 2048
I = 1024
E = 32
NS = 2
TOP_K = 6
SCALE = 1.0
NCORES = 8
SLOTS = 4          # routed experts per core (shared expert is slot SLOTS)
TSH = 512          # shared-expert tokens per core (T / 4; 2-way expert split)
KH = H // 128      # 16 k-tiles over H
KI = I // 128      # 8 k-tiles over I
PH = H // 256      # 8 double-row pairs over H
PI = I // 256      # 4 double-row pairs over I
BF16 = mybir.dt.bfloat16
F32 = mybir.dt.float32
FP8 = mybir.dt.float8e4
NP_FP8 = mybir.dt.np(FP8)

# power-of-2 fp8 scales (descales are folded into sigmoid scale / gates).
# fp8e4 here is IEEE e4m3 (max finite 240): z = sigmoid(g) * u carries
# S_WU * S_X = 32x and must stay well under 240 when cast to fp8.
S_X = 8.0          # tokens
S_WG = 8.0         # gate weights
S_WU = 4.0         # up weights
S_WD = 32.0        # down weights
DESCALE_GATE = 1.0 / (S_WG * S_X)                    # on sigmoid input
S_Y = 64.0         # fp8 y-output scale (divided out on host)
DESCALE_Y = S_Y / (S_WU * S_X * S_WD)                # folded into gates

_PROGRAM_CACHE: dict = {}


def _to_bf16(a: np.ndarray) -> np.ndarray:
    """f32 -> bf16 with round-to-nearest-even (fast uint trick)."""
    a = np.ascontiguousarray(a, dtype=np.float32)
    u = a.view(np.uint32)
    r = (u + np.uint32(0x7FFF) + ((u >> np.uint32(16)) & np.uint32(1))) >> np.uint32(16)
    return r.astype(np.uint16).view(ml_dtypes.bfloat16)


def _fp8_pm(a: np.ndarray, scale: float) -> np.ndarray:
    """[K, N] f32 -> [128, K/256, 2, N] fp8e4, partition-major DoubleRow.

    Partition q, pair p, sub-row r holds contraction row k = 256p + 128r + q.
    """
    K, N = a.shape
    q = (np.asarray(a, np.float32) * scale).reshape(K // 256, 2, 128, N)
    return np.ascontiguousarray(q.transpose(2, 0, 1, 3)).astype(NP_FP8)


def _bf16_pm(a: np.ndarray) -> np.ndarray:
    """[K, N] f32 -> [128, K/128, N] bf16, partition-major."""
    K, N = a.shape
    q = _to_bf16(a).reshape(K // 128, 128, N)
    return np.ascontiguousarray(q.transpose(1, 0, 2))


def _route(flat: np.ndarray, Wr: np.ndarray):
    """Host router: softmax over experts, exact top-k gate mask."""
    logits = flat.astype(np.float32) @ Wr.astype(np.float32)
    m = logits.max(axis=-1, keepdims=True)
    p = np.exp(logits - m)
    p /= p.sum(axis=-1, keepdims=True)
    T = p.shape[0]
    idx = np.argpartition(-p, TOP_K - 1, axis=-1)[:, :TOP_K]
    gates = np.zeros((T, E), np.float32)
    rows = np.arange(T)[:, None]
    gates[rows, idx] = p[rows, idx] * SCALE
    return gates


def _assign_experts(tok_idx):
    """Deal experts into (core, slot) balancing per-core token totals.

    Experts with more than 512 tokens (the PSUM-bank N limit) are split into
    pseudo-experts with disjoint token chunks, so slot capacity never exceeds
    512. Slot s holds the pseudo-experts ranked [8s, 8s+8) by token count;
    within a slot the largest goes to the least-loaded core. Returns
    (assign, caps, chunks) where chunks[j] = (expert, token_index_array) and
    assign[core][slot] indexes into chunks (-1 = empty).
    """
    chunks = []
    for e, ix in enumerate(tok_idx):
        for off in range(0, max(len(ix), 1), 512):
            chunks.append((e, ix[off:off + 512]))
    while len(chunks) % NCORES:
        chunks.append((0, np.zeros(0, np.int32)))
    counts = np.array([len(ix) for _, ix in chunks], np.int64)
    n_slots = len(chunks) // NCORES
    order = np.argsort(-counts, kind="stable")
    assign = [[-1] * n_slots for _ in range(NCORES)]
    load = np.zeros(NCORES, np.int64)
    caps = []
    for s in range(n_slots):
        group = list(order[s * NCORES:(s + 1) * NCORES])
        caps.append(int(counts[group].max()) if group else 0)
        for j in group:  # descending count; give to least-loaded core
            c = int(np.argmin(load))
            assign[c][s] = int(j)
            load[c] += counts[j]
    caps = [min(512, max(64, -(-c // 8) * 8)) for c in caps]
    return assign, caps, chunks


def build_program(caps, loop_reps=None, parts="all", unroll=1):
    """Build the per-core Bass program for the given slot capacities.

    caps includes the shared slot LAST (always TSH; mixed-precision path).
    loop_reps: if set, wrap the whole body in a device-side For_i loop —
    used by the test harness to amplify exec time above dispatch overhead.
    unroll: kernel bodies emitted per loop iteration (pipelined into each
    other), amortizing the For_i all-engine barrier in timing runs.
    """
    caps = tuple(int(c) for c in caps)
    key = (caps, loop_reps, parts, unroll)
    if key in _PROGRAM_CACHE:
        return _PROGRAM_CACHE[key]

    nslots = len(caps)
    shared_s = nslots - 1
    assert caps[shared_s] == TSH
    rtot = sum(caps[:-1])
    roff = [sum(caps[:s]) for s in range(nslots - 1)]

    nc = bacc.Bacc("TRN2", target_bir_lowering=False, debug=False)

    xg_d, wg_d, wu_d, wd_d, y_d = [], [], [], [], []
    for s in range(nslots - 1):
        C = caps[s]
        xg_d.append(nc.dram_tensor(f"xg{s}", [128, PH, 2, C], FP8, kind="ExternalInput"))
        wg_d.append(nc.dram_tensor(f"wg{s}", [128, PH, 2, I], FP8, kind="ExternalInput"))
        wu_d.append(nc.dram_tensor(f"wu{s}", [128, PH, 2, I], FP8, kind="ExternalInput"))
        wd_d.append(nc.dram_tensor(f"wd{s}", [128, PI, 2, H], FP8, kind="ExternalInput"))
        y_d.append(nc.dram_tensor(f"y{s}", [128, KH, C], FP8, kind="ExternalOutput"))
    gall_d = nc.dram_tensor("gall", [1, rtot], BF16, kind="ExternalInput")
    # shared slot: fp8 gate inputs + bf16 up/down
    xg4_d = nc.dram_tensor("xg4", [128, PH, 2, TSH], FP8, kind="ExternalInput")
    wg4_d = nc.dram_tensor("wg4", [128, PH, 2, I], FP8, kind="ExternalInput")
    wu4_d = nc.dram_tensor("wu4", [128, PH, 2, I], FP8, kind="ExternalInput")
    wd4_d = nc.dram_tensor("wd4", [128, KI, H], BF16, kind="ExternalInput")
    ys_d = nc.dram_tensor("ys", [128, KH, TSH], BF16, kind="ExternalOutput")

    DR = mybir.MatmulPerfMode.DoubleRow
    SIG = mybir.ActivationFunctionType.Sigmoid

    with tile.TileContext(nc) as tc:
        with (
            tc.tile_pool(name="w", bufs=6) as wpool,
            tc.tile_pool(name="xg", bufs=2) as xpool,
            tc.tile_pool(name="gb", bufs=1) as gbpool,
            tc.tile_pool(name="sg", bufs=2) as sgpool,
            tc.tile_pool(name="z", bufs=2) as zpool,
            tc.tile_pool(name="o", bufs=2) as opool,
            tc.tile_pool(name="pg", bufs=2, space="PSUM") as pgpool,
            tc.tile_pool(name="pu", bufs=2, space="PSUM") as pupool,
            tc.tile_pool(name="py", bufs=4, space="PSUM") as pypool,
            ExitStack() as stack,
        ):
            if loop_reps is not None:
                stack.enter_context(tc.For_i(0, loop_reps, 1))

            gb_all = {}

            def load_gb():
                # on the scalar queue: the 128-way replicate DMA must not
                # delay the sync queue's weight stream
                t = gbpool.tile([128, rtot], BF16, tag="gb", name="gb_all")
                nc.scalar.dma_start(t[:], gall_d[:].partition_broadcast(128))
                gb_all["t"] = t

            def dma_A(s, u, first=False):
                """Issue slot s's gate/up-phase loads; return slot state.

                For the first slot of an iteration, wg rides the (empty)
                scalar queue so it loads concurrently with xg on sync.
                """
                weng = nc.scalar if first else nc.sync
                st = {"s": s, "C": caps[s], "u": u}
                sfx = f"{s}_u{u}"
                if s == shared_s:
                    xg_t = xpool.tile([128, PH, 2, TSH], FP8, tag="xg", name=f"xg_t{sfx}")
                    nc.sync.dma_start(xg_t[:], xg4_d[:])
                    wg_t = wpool.tile([128, PH, 2, I], FP8, tag="w", name=f"wg_t{sfx}")
                    weng.dma_start(wg_t[:], wg4_d[:])
                    wu_t = wpool.tile([128, PH, 2, I], FP8, tag="w", name=f"wu_t{sfx}")
                    nc.sync.dma_start(wu_t[:], wu4_d[:])
                    st.update(xg=xg_t, wg=wg_t, wu=wu_t)
                else:
                    C = caps[s]
                    xg_t = xpool.tile([128, PH, 2, C], FP8, tag="xg", name=f"xg_t{sfx}")
                    nc.sync.dma_start(xg_t[:], xg_d[s][:])
                    wg_t = wpool.tile([128, PH, 2, I], FP8, tag="w", name=f"wg_t{sfx}")
                    weng.dma_start(wg_t[:], wg_d[s][:])
                    wu_t = wpool.tile([128, PH, 2, I], FP8, tag="w", name=f"wu_t{sfx}")
                    nc.sync.dma_start(wu_t[:], wu_d[s][:])
                    st.update(xg=xg_t, wg=wg_t, wu=wu_t)
                return st

            def dma_B(s, st):
                """Issue slot s's down-phase weight load."""
                sfx = f"{s}_u{st['u']}"
                if s == shared_s:
                    wd_t = [wpool.tile([128, KI // 2, H], BF16, tag="w",
                                       name=f"wd4_{hf}_{sfx}") for hf in range(2)]
                    for hf in range(2):
                        nc.sync.dma_start(wd_t[hf][:], wd4_d[:, 4 * hf:4 * hf + 4])
                else:
                    wd_t = wpool.tile([128, PI, 2, H], FP8, tag="w", name=f"wd_t{sfx}")
                    nc.sync.dma_start(wd_t[:], wd_d[s][:])
                st["wd"] = wd_t

            def groups_A(s, st):
                """16 thunks: 8 gate-projection + 8 up-projection PSUM groups."""
                C = st["C"]
                sfx = f"{s}_u{st['u']}"
                sg = sgpool.tile([128, KI, C], BF16, tag="sg", name=f"sg{sfx}")
                zdt = BF16 if s == shared_s else FP8
                z = zpool.tile([128, KI, C], zdt, tag="z", name=f"z{sfx}")
                st["sg"], st["z"] = sg, z

                def gate(m):
                    pg = pgpool.tile([128, C], F32, tag="pg", name=f"pg{sfx}_{m}")
                    for p in range(PH):
                        nc.tensor.matmul(pg[:], st["wg"][:, p, :, m * 128:(m + 1) * 128],
                                         st["xg"][:, p], start=(p == 0),
                                         stop=(p == PH - 1), perf_mode=DR)
                    nc.scalar.activation(sg[:, m, :], pg[:], SIG, scale=DESCALE_GATE)

                def up(m):
                    pu = pupool.tile([128, C], F32, tag="pu", name=f"pu{sfx}_{m}")
                    for p in range(PH):
                        nc.tensor.matmul(pu[:],
                                         st["wu"][:, p, :, m * 128:(m + 1) * 128],
                                         st["xg"][:, p], start=(p == 0),
                                         stop=(p == PH - 1), perf_mode=DR)
                    nc.vector.tensor_mul(z[:, m, :], sg[:, m, :], pu[:])

                return ([lambda m=m: gate(m) for m in range(KI)]
                        + [lambda m=m: up(m) for m in range(KI)])

            def groups_B(s, st):
                """16 thunks: down-projection PSUM groups; last emits y store."""
                C = st["C"]
                sfx = f"{s}_u{st['u']}"
                ydt = BF16 if s == shared_s else FP8
                ot = opool.tile([128, KH, C], ydt, tag="o", name=f"ot{sfx}")

                def down(h):
                    py = pypool.tile([128, C], F32, tag="py", name=f"py{sfx}_{h}")
                    if s == shared_s:
                        for j in range(KI):
                            nc.tensor.matmul(py[:],
                                             st["wd"][j // 4][:, j % 4, h * 128:(h + 1) * 128],
                                             st["z"][:, j, :], start=(j == 0),
                                             stop=(j == KI - 1))
                        nc.vector.tensor_copy(ot[:, h, :], py[:])
                    else:
                        for p in range(PI):
                            nc.tensor.matmul(py[:],
                                             st["wd"][:, p, :, h * 128:(h + 1) * 128],
                                             st["z"][:, 2 * p:2 * p + 2, :],
                                             start=(p == 0), stop=(p == PI - 1),
                                             perf_mode=DR)
                        nc.vector.tensor_mul(ot[:, h, :], py[:],
                                             gb_all["t"][:, roff[s]:roff[s] + C])
                    yd = ys_d if s == shared_s else y_d[s]
                    if h == KH // 2 - 1:
                        nc.scalar.dma_start(yd[:, :KH // 2], ot[:, :KH // 2])
                    elif h == KH - 1:
                        nc.scalar.dma_start(yd[:, KH // 2:], ot[:, KH // 2:])

                return [lambda h=h: down(h) for h in range(KH)]

            if parts == "routed":
                order = list(range(nslots - 1))
            elif parts == "shared":
                order = [shared_s]
            else:
                # shared second: its long bf16 down-phase zips with routed
                # gate/up work, and a short routed slot forms the tail
                order = [0, shared_s] + list(range(1, nslots - 1))

            if parts == "tiny":
                tt = sgpool.tile([128, 64], BF16, tag="sg", name="tiny")
                nc.vector.memset(tt[:], 0.0)
            else:
                pending = None
                for u in range(unroll):
                    for i, s in enumerate(order):
                        st = dma_A(s, u, first=(u == 0 and i == 0))
                        dma_B(s, st)
                        if u == 0 and i == 0:
                            load_gb()
                        A = groups_A(s, st)
                        if pending is None:
                            for t in A:
                                t()
                        else:
                            for tb, ta in zip(pending, A):
                                tb()
                                ta()
                        pending = groups_B(s, st)
                for t in pending:
                    t()

    nc.compile()
    _PROGRAM_CACHE[key] = nc
    return nc


def prepare(x, Wr, Wg_s, Wu_s, Wd_s, Wg, Wu, Wd):
    """Host-side routing, sharding and fp8/bf16 packing. Returns (nc, in_maps, meta)."""
    flat = np.ascontiguousarray(x, np.float32).reshape(-1, H)
    T = flat.shape[0]
    assert T == 4 * TSH

    gates = _route(flat, Wr)
    tok_idx = [np.nonzero(gates[:, e])[0].astype(np.int32) for e in range(E)]
    assign, rcaps, chunks = _assign_experts(tok_idx)
    caps = rcaps + [TSH]

    nc = build_program(caps)

    xT = np.ascontiguousarray(flat.T)          # [H, T] f32
    xg_sh = [_fp8_pm(xT[:, p * TSH:(p + 1) * TSH], S_X) for p in range(4)]
    wg_sh = [_fp8_pm(np.asarray(Wg_s[e]), S_WG) for e in range(NS)]
    wu_sh = [_fp8_pm(np.asarray(Wu_s[e]), S_WU) for e in range(NS)]
    wd_sh = [_bf16_pm(np.asarray(Wd_s[e])) for e in range(NS)]
    wg_r, wu_r, wd_r = {}, {}, {}

    in_maps = []
    for c in range(NCORES):
        half, part = divmod(c, 4)
        im = {"xg4": xg_sh[part], "wg4": wg_sh[half],
              "wu4": wu_sh[half], "wd4": wd_sh[half]}
        gall = np.zeros((1, sum(rcaps)), np.float32)
        off = 0
        for s in range(SLOTS):
            e, ix = chunks[assign[c][s]]
            C = caps[s]
            xg = np.zeros((H, C), np.float32)
            xg[:, :len(ix)] = xT[:, ix]
            im[f"xg{s}"] = _fp8_pm(xg, S_X)
            gall[0, off:off + len(ix)] = gates[ix, e] * DESCALE_Y
            off += C
            if e not in wg_r:
                wg_r[e] = _fp8_pm(np.asarray(Wg[e]), S_WG)
                wu_r[e] = _fp8_pm(np.asarray(Wu[e]), S_WU)
                wd_r[e] = _fp8_pm(np.asarray(Wd[e]), S_WD)
            im[f"wg{s}"] = wg_r[e]
            im[f"wu{s}"] = wu_r[e]
            im[f"wd{s}"] = wd_r[e]
        im["gall"] = _to_bf16(gall)
        in_maps.append(im)

    meta = {"assign": assign, "caps": caps, "chunks": chunks,
            "flat": flat, "shape": x.shape}
    return nc, in_maps, meta


def postprocess(results, meta):
    """Scatter-add per-expert outputs + shared shards + residual."""
    flat = meta["flat"]
    caps = meta["caps"]
    out = flat.copy()
    inv_sy = 1.0 / S_Y
    for c in range(NCORES):
        part = c % 4
        ysh = results[c]["ys"].astype(np.float32)             # [128, KH, TSH]
        sh = ysh.transpose(1, 0, 2).reshape(H, TSH)
        out[part * TSH:(part + 1) * TSH] += sh.T * (1.0 / (S_WU * S_X))
        for s in range(SLOTS):
            _, ix = meta["chunks"][meta["assign"][c][s]]
            if len(ix) == 0:
                continue
            Y = results[c][f"y{s}"].astype(np.float32)        # [128, KH, C]
            Yf = Y.transpose(1, 0, 2).reshape(H, caps[s])
            out[ix] += Yf[:, :len(ix)].T * inv_sy
    return out.reshape(meta["shape"]).astype(np.float32, copy=False)


def kernel(x, Wr, Wg_s, Wu_s, Wd_s, Wg, Wu, Wd):
    nc, in_maps, meta = prepare(x, Wr, Wg_s, Wu_s, Wd_s, Wg, Wu, Wd)
    last_err = None
    for _ in range(3):  # the tunneled device occasionally drops a run
        try:
            res = run_bass_kernel_spmd(nc, in_maps, list(range(NCORES)))
            return postprocess(res.results, meta)
        except Exception as err:  # noqa: BLE001
            last_err = err
    raise last_err
